# revision 14
# baseline (speedup 1.0000x reference)
"""Causal single-head attention (B=4, S=2048, D=1024) on 8 Trainium2 cores.

Sharding: 8 cores = (batch b, stripe-set eta). Core (b, eta) owns four
interleaved key stripes of 256 rows at global offsets 512k + 256*eta
(k = 0..3) of batch b, stored locally stripe-major (local key
ell in [256k, 256k+256) -> global 512k + 256*eta + ell%256). Queries are
fed "aligned" with base beta = 256*eta: query col c corresponds to global
row beta + c. Then the causal condition for local key tile kt vs query
chunk rc is c >= 512*(kt//2) + 128*(kt%2) + x — identical on every core,
so one SPMD program serves both stripe sets with a purely compile-time
block mask, and score blocks with kt >= 2*(rc+1) are skipped outright
(20 of 32 blocks kept vs 28 for a half-split). Cols past the sequence end
(eta=1, c >= 1792) compute junk that the host discards.

Softmax uses no max-subtraction (logits are O(1) for this problem:
|score/32| < ~4), so per-core partials are just num = exp(S)·V and
l = sum(exp(S)); the host merges halves with num/den addition and one
divide. This is mathematically identical to the reference softmax.

The Q and K projections are folded away algebraically: scores =
x_kv (Wk^T Wq) x^T, with G = Wk^T Wq precomputed on the HOST (weights
only). On-chip: M^T = G^T x_kv^T costs 1024*D^2 MACs — replacing the
2048*D^2 Q projection and 1024*D^2 K projection outright.

On-chip layout (all matmul operands fp16 — same PE rate as bf16 with 3
extra mantissa bits; every tensor here is O(10) so fp16 range is safe —
fp32 PSUM accumulation):
  xt = x_core^T [D=1024, NQ=2048]; xkv = stripe-gathered key cols of xt.
    MT = g.T @ xkv   [i', j]      V = xkv.T @ wvT   [j, d]
    ST = MT.T @ xt   (scores transposed: partition=key, free=query)
    PT = exp(ST/32) causally zeroed. PV runs with PT sub-blocks stationary
    and V moving: O[c, d] += PT_sub.T @ V, and the denominator comes free
    as an N=1 matmul on the same stationary: l = PT_sub.T @ 1s.
  Outputs per core: o [NQ, D] fp32 numerator, ls [128, 16] fp32 denom
  (query col c lives at ls[c % 128, c // 128]).
"""

import sys

sys.path.insert(0, "/opt/trn_rl_repo")

from contextlib import ExitStack

import ml_dtypes
import numpy as np

import concourse.bass as bass  # noqa: F401  (engine types resolve via bacc)
import concourse.mybir as mybir
import concourse.tile as tile
from concourse import bacc, bass_utils
from concourse.bass import ts

F16 = mybir.dt.float16  # same PE speed as bf16, 3 more mantissa bits
F32 = mybir.dt.float32

P = 128            # partitions
D = 1024           # model dim (d_in == d_out)
NQ = 2048          # query slots per core
NK = 1024          # keys per core
RC = 512           # query-chunk (matmul moving-dim) size
N_RC = NQ // RC    # 4
N_KT = NK // P     # 8 key tiles
N_IB = D // P      # 8 contraction blocks
SCALE = 1.0 / 32.0 # 1/sqrt(D)

N_CORES = 8
B, S = 4, 2048
STRIPE = 256


def _kept_kts(rc):
    # key tile kt (stripe k = kt//2) is visible to query chunk rc iff some
    # col c in [rc*512, rc*512+512) has c >= 512*(kt//2) + 128*(kt%2).
    return [kt for kt in range(N_KT) if kt < 2 * (rc + 1)]


def _mask_base(rc, kt):
    # keep when  y + 512*rc >= x + 512*(kt//2) + 128*(kt%2)
    return RC * rc - RC * (kt // 2) - P * (kt % 2)


def _emit(nc, tc, xt, g, wvt, ot, ls):
    with ExitStack() as ctx:
        sb = ctx.enter_context(tc.tile_pool(name="sb", bufs=1))
        pts = ctx.enter_context(tc.tile_pool(name="pts", bufs=1))
        outp = ctx.enter_context(tc.tile_pool(name="outp", bufs=4))
        ps = ctx.enter_context(tc.tile_pool(name="ps", bufs=7, space="PSUM"))
        psl = ctx.enter_context(tc.tile_pool(name="psl", bufs=1, space="PSUM"))

        ones = sb.tile([P, 1], F16, tag="ones", name="ones")
        nc.vector.memset(ones, 1.0)

        # HAM warm-up: ~6us of dummy matmuls that need no DMA, issued while
        # the NEFF preamble + first input loads run. They lift the PE clock
        # gate from 1.2 to 2.4 GHz before real matmuls arrive. The result is
        # parked in l_sb, whose every column is overwritten later.
        warm = sb.tile([P, RC], F16, tag="warm", name="warm")
        nc.vector.memset(warm, 0.0)
        l_sb = sb.tile([P, N_RC * 4], F32, tag="lsb", name="lsb")
        acc_w = ps.tile([P, RC], F32, tag="mm", name="acc_w")
        N_WARM = 12
        for w in range(N_WARM):
            nc.tensor.matmul(acc_w, lhsT=warm[:, 0:P], rhs=warm,
                             start=(w == 0), stop=(w == N_WARM - 1))
        nc.vector.tensor_copy(l_sb, acc_w[:, 0:N_RC * 4])

        # ---- input loads ----
        # Emission order = consumption order, so the first KT matmul can
        # start ~1.5us in (needs only wk[0] + xt[0] low half) instead of
        # stalling on the full 10MB input load.
        xt_sb = [sb.tile([P, NQ], F16, tag=f"xt{i}", name=f"xt{i}")
                 for i in range(N_IB)]
        g_sb = [sb.tile([P, D], F16, tag=f"g{i}", name=f"g{i}")
                for i in range(N_IB)]
        wv_sb = [sb.tile([P, D], F16, tag=f"wv{i}", name=f"wv{i}")
                 for i in range(N_IB)]
        xkv_sb = [sb.tile([P, NK], F16, tag=f"xkv{i}", name=f"xkv{i}")
                  for i in range(N_IB)]
        for i in range(N_IB):
            nc.sync.dma_start(out=g_sb[i], in_=g[ts(i, P), :])
            # gather the 4 key stripes (cols 512k..512k+256 of xt) into a
            # dense [128, 1024] stripe-major kv tile
            nc.sync.dma_start(
                out=xkv_sb[i].rearrange("p (k c) -> p k c", c=256),
                in_=xt[ts(i, P), :].rearrange("p (k c) -> p k c", c=512)[:, :, 0:256])
        for i in range(N_IB):
            nc.sync.dma_start(out=wv_sb[i], in_=wvt[ts(i, P), :])
        for i in range(N_IB):
            nc.sync.dma_start(out=xt_sb[i][:, 0:NK], in_=xt[ts(i, P), 0:NK])
        for i in range(N_IB):
            nc.sync.dma_start(out=xt_sb[i][:, NK:NQ], in_=xt[ts(i, P), NK:NQ])

        # ---- projections ----
        # i-major emission in batches of 4 PSUM groups: each arriving DMA
        # block immediately feeds 4 matmuls, and consecutive matmuls with
        # the same stationary operand sit adjacent in the PE stream.
        def proj_phase(groups, dst, lhs_of, rhs_of):
            for gb in range(0, len(groups), 4):
                batch = groups[gb:gb + 4]
                accs = [ps.tile([P, RC], F32, tag="mm", name="acc_p")
                        for _ in batch]
                for i in range(N_IB):
                    for a, g in zip(accs, batch):
                        nc.tensor.matmul(a, lhsT=lhs_of(i, g),
                                         rhs=rhs_of(i, g),
                                         start=(i == 0), stop=(i == N_IB - 1))
                for a, g in zip(accs, batch):
                    nc.vector.tensor_copy(dst(g), a)

        mt_sb = [sb.tile([P, NK], F16, tag=f"mt{o}", name=f"mt{o}")
                 for o in range(N_IB)]
        proj_phase([(o, jc) for o in range(N_IB) for jc in range(NK // RC)],
                   dst=lambda gr: mt_sb[gr[0]][:, ts(gr[1], RC)],
                   lhs_of=lambda i, gr: g_sb[i][:, ts(gr[0], P)],
                   rhs_of=lambda i, gr: xkv_sb[i][:, ts(gr[1], RC)])

        v_sb = [sb.tile([P, D], F16, tag=f"vj{j}", name=f"vj{j}")
                for j in range(N_KT)]
        proj_phase([(j, dc) for j in range(N_KT) for dc in range(D // RC)],
                   dst=lambda gr: v_sb[gr[0]][:, ts(gr[1], RC)],
                   lhs_of=lambda i, gr: xkv_sb[i][:, ts(gr[0], P)],
                   rhs_of=lambda i, gr: wv_sb[i][:, ts(gr[1], RC)])

        # ---- attention ----
        # ST is emitted kt-major so the stationary K^T block is reused by
        # consecutive matmuls; PV uses P^T sub-blocks as the stationary
        # operand (V moving), which makes the row-sum l an extra N=1 matmul
        # on an already-loaded stationary and yields output in natural
        # [query, d] orientation.
        pt_tiles = {}

        def kept_rcs(kt):
            return [rc for rc in range(N_RC) if kt in _kept_kts(rc)]

        def emit_st(kt):
            rcs = kept_rcs(kt)
            accs = {rc: ps.tile([P, RC], F32, tag="mm", name="acc_st")
                    for rc in rcs}
            for o in range(N_IB):
                for rc in rcs:
                    nc.tensor.matmul(accs[rc],
                                     lhsT=mt_sb[o][:, ts(kt, P)],
                                     rhs=xt_sb[o][:, ts(rc, RC)],
                                     start=(o == 0), stop=(o == N_IB - 1))
            for rc in rcs:
                pt = pts.tile([P, RC], F16, tag=f"pt{kt}_{rc}",
                              name=f"pt{kt}_{rc}")
                nc.scalar.activation(pt, accs[rc],
                                     mybir.ActivationFunctionType.Exp,
                                     scale=SCALE)
                base = _mask_base(rc, kt)
                if base < P - 1:  # tile straddles the causal diagonal
                    nc.gpsimd.affine_select(
                        out=pt, in_=pt,
                        compare_op=mybir.AluOpType.is_ge, fill=0.0,
                        base=base, channel_multiplier=-1, pattern=[[1, RC]])
                pt_tiles[(kt, rc)] = pt

        def emit_pv(rc):
            kts = _kept_kts(rc)
            last = len(kts) - 1
            for rsub in range(RC // P):
                pos = [ps.tile([P, RC], F32, tag="mm", name="acc_pv")
                       for _ in range(D // RC)]
                pl = psl.tile([P, 1], F32, tag="lp", name="lp")
                for n, kt in enumerate(kts):
                    lhs = pt_tiles[(kt, rc)][:, ts(rsub, P)]
                    for dc, po in enumerate(pos):
                        nc.tensor.matmul(po, lhsT=lhs,
                                         rhs=v_sb[kt][:, ts(dc, RC)],
                                         start=(n == 0), stop=(n == last))
                    nc.tensor.matmul(pl, lhsT=lhs, rhs=ones,
                                     start=(n == 0), stop=(n == last))
                row = rc * RC + rsub * P
                for dc, po in enumerate(pos):
                    o_sb = outp.tile([P, RC], F32, tag="osb", name="osb")
                    nc.vector.tensor_copy(o_sb, po)
                    nc.sync.dma_start(out=ot[row:row + P, ts(dc, RC)],
                                      in_=o_sb)
                nc.vector.tensor_copy(l_sb[:, rc * 4 + rsub:rc * 4 + rsub + 1],
                                      pl)

        # software-pipelined emission: PV(rc) right after its last key tile
        emit_st(0)
        emit_st(1)
        emit_pv(0)
        emit_st(2)
        emit_st(3)
        emit_pv(1)
        emit_st(4)
        emit_st(5)
        emit_pv(2)
        emit_st(6)
        emit_st(7)
        emit_pv(3)
        nc.sync.dma_start(out=ls, in_=l_sb)


_NC_CACHE = {}


def _get_nc():
    if "nc" not in _NC_CACHE:
        nc = bacc.Bacc("TRN2", target_bir_lowering=False, debug=False,
                       enable_asserts=False, num_devices=N_CORES)
        xt = nc.dram_tensor("xt", [D, NQ], F16, kind="ExternalInput").ap()
        g = nc.dram_tensor("g", [D, D], F16, kind="ExternalInput").ap()
        wvt = nc.dram_tensor("wvt", [D, D], F16, kind="ExternalInput").ap()
        ot = nc.dram_tensor("ot", [NQ, D], F32, kind="ExternalOutput").ap()
        ls = nc.dram_tensor("ls", [P, N_RC * 4], F32, kind="ExternalOutput").ap()
        with tile.TileContext(nc) as tc:
            _emit(nc, tc, xt, g, wvt, ot, ls)
        nc.compile()
        _NC_CACHE["nc"] = nc
    return _NC_CACHE["nc"]


def make_in_maps(x, w_query, w_key, w_value):
    bf = np.float16
    wq32 = np.asarray(w_query, dtype=np.float32)
    wk32 = np.asarray(w_key, dtype=np.float32)
    # fold the Q and K projections: scores = x_kv (Wk^T Wq) x^T
    g_np = np.ascontiguousarray(wk32.T @ wq32).astype(bf)
    wvt = np.ascontiguousarray(np.asarray(w_value).T).astype(bf)
    in_maps = []
    for c in range(N_CORES):
        b, eta = c // 2, c % 2
        rows = (np.arange(NQ) + eta * STRIPE) % S  # cols past S wrap to junk
        xt_np = np.ascontiguousarray(np.asarray(x)[b, rows].T).astype(bf)
        in_maps.append({"xt": xt_np, "g": g_np, "wvt": wvt})
    return in_maps


def merge_outputs(results):
    num = np.zeros((B, S, D), np.float32)
    den = np.zeros((B, S), np.float32)
    for c in range(N_CORES):
        b, eta = c // 2, c % 2
        otc = np.asarray(results[c]["ot"])   # [NQ, D]
        # ls[p, col] holds l for query col c = col*128 + p
        lc = np.asarray(results[c]["ls"]).T.reshape(NQ)
        beta = eta * STRIPE
        nvalid = S - beta
        num[b, beta:] += otc[:nvalid]
        den[b, beta:] += lc[:nvalid]
    return (num / den[:, :, None]).astype(np.float32)


def kernel(x, w_query, w_key, w_value, _trace=False):
    nc = _get_nc()
    in_maps = make_in_maps(x, w_query, w_key, w_value)
    res = bass_utils.run_bass_kernel_spmd(
        nc, in_maps, core_ids=list(range(N_CORES)), trace=_trace)
    out = merge_outputs(res.results)
    if _trace:
        kernel.last_result = res
    return out


# revision 15
# speedup vs baseline: 1.0097x; 1.0097x over previous
"""Causal single-head attention (B=4, S=2048, D=1024) on 8 Trainium2 cores.

Sharding: 8 cores = (batch b, stripe-set eta). Core (b, eta) owns four
interleaved key stripes of 256 rows at global offsets 512k + 256*eta
(k = 0..3) of batch b, stored locally stripe-major (local key
ell in [256k, 256k+256) -> global 512k + 256*eta + ell%256). Queries are
fed "aligned" with base beta = 256*eta: query col c corresponds to global
row beta + c. Then the causal condition for local key tile kt vs query
chunk rc is c >= 512*(kt//2) + 128*(kt%2) + x — identical on every core,
so one SPMD program serves both stripe sets with a purely compile-time
block mask, and score blocks with kt >= 2*(rc+1) are skipped outright
(20 of 32 blocks kept vs 28 for a half-split). Cols past the sequence end
(eta=1, c >= 1792) compute junk that the host discards.

Softmax uses no max-subtraction (logits are O(1) for this problem:
|score/32| < ~4), so per-core partials are just num = exp(S)·V and
l = sum(exp(S)); the host merges halves with num/den addition and one
divide. This is mathematically identical to the reference softmax.

The Q and K projections are folded away algebraically: scores =
x_kv (Wk^T Wq) x^T, with G = Wk^T Wq precomputed on the HOST (weights
only). On-chip: M^T = G^T x_kv^T costs 1024*D^2 MACs — replacing the
2048*D^2 Q projection and 1024*D^2 K projection outright.

On-chip layout (all matmul operands fp16 — same PE rate as bf16 with 3
extra mantissa bits; every tensor here is O(10) so fp16 range is safe —
fp32 PSUM accumulation):
  xt = x_core^T [D=1024, NQ=2048]; xkv = stripe-gathered key cols of xt.
    MT = g.T @ xkv   [i', j]      V = xkv.T @ wvT   [j, d]
    ST = MT.T @ xt   (scores transposed: partition=key, free=query)
    PT = exp(ST/32) causally zeroed. PV runs with PT sub-blocks stationary
    and V moving: O[c, d] += PT_sub.T @ V, and the denominator comes free
    as an N=1 matmul on the same stationary: l = PT_sub.T @ 1s.
  Outputs per core: o [NQ, D] fp32 numerator, ls [128, 16] fp32 denom
  (query col c lives at ls[c % 128, c // 128]).
"""

import sys

sys.path.insert(0, "/opt/trn_rl_repo")

from contextlib import ExitStack

import ml_dtypes
import numpy as np

import concourse.bass as bass  # noqa: F401  (engine types resolve via bacc)
import concourse.mybir as mybir
import concourse.tile as tile
from concourse import bacc, bass_utils
from concourse.bass import ts

F16 = mybir.dt.float16  # same PE speed as bf16, 3 more mantissa bits
F32 = mybir.dt.float32

P = 128            # partitions
D = 1024           # model dim (d_in == d_out)
NQ = 2048          # query slots per core
NK = 1024          # keys per core
RC = 512           # query-chunk (matmul moving-dim) size
N_RC = NQ // RC    # 4
N_KT = NK // P     # 8 key tiles
N_IB = D // P      # 8 contraction blocks
SCALE = 1.0 / 32.0 # 1/sqrt(D)

N_CORES = 8
B, S = 4, 2048
STRIPE = 256


def _kept_kts(rc):
    # key tile kt (stripe k = kt//2) is visible to query chunk rc iff some
    # col c in [rc*512, rc*512+512) has c >= 512*(kt//2) + 128*(kt%2).
    return [kt for kt in range(N_KT) if kt < 2 * (rc + 1)]


def _mask_base(rc, kt):
    # keep when  y + 512*rc >= x + 512*(kt//2) + 128*(kt%2)
    return RC * rc - RC * (kt // 2) - P * (kt % 2)


def _emit(nc, tc, xt, g, wvt, ot, ls):
    with ExitStack() as ctx:
        sb = ctx.enter_context(tc.tile_pool(name="sb", bufs=1))
        pts = ctx.enter_context(tc.tile_pool(name="pts", bufs=1))
        outp = ctx.enter_context(tc.tile_pool(name="outp", bufs=4))
        ps = ctx.enter_context(tc.tile_pool(name="ps", bufs=7, space="PSUM"))
        psl = ctx.enter_context(tc.tile_pool(name="psl", bufs=1, space="PSUM"))

        ones = sb.tile([P, 1], F16, tag="ones", name="ones")
        nc.vector.memset(ones, 1.0)

        # HAM warm-up: ~6us of dummy matmuls that need no DMA, issued while
        # the NEFF preamble + first input loads run. They lift the PE clock
        # gate from 1.2 to 2.4 GHz before real matmuls arrive. The result is
        # parked in l_sb, whose every column is overwritten later.
        warm = sb.tile([P, RC], F16, tag="warm", name="warm")
        nc.vector.memset(warm, 0.0)
        l_sb = sb.tile([P, N_RC * 4], F32, tag="lsb", name="lsb")
        acc_w = ps.tile([P, RC], F32, tag="mm", name="acc_w")
        N_WARM = 12
        for w in range(N_WARM):
            nc.tensor.matmul(acc_w, lhsT=warm[:, 0:P], rhs=warm,
                             start=(w == 0), stop=(w == N_WARM - 1))
        nc.vector.tensor_copy(l_sb, acc_w[:, 0:N_RC * 4])

        # ---- input loads ----
        # Emission order = consumption order: MT needs only g + xkv
        # (4MB), so PE compute starts while wv/xt are still in flight.
        xt_sb = [sb.tile([P, NQ], F16, tag=f"xt{i}", name=f"xt{i}")
                 for i in range(N_IB)]
        g_sb = [sb.tile([P, D], F16, tag=f"g{i}", name=f"g{i}")
                for i in range(N_IB)]
        wv_sb = [sb.tile([P, D], F16, tag=f"wv{i}", name=f"wv{i}")
                 for i in range(N_IB)]
        xkv_sb = [sb.tile([P, NK], F16, tag=f"xkv{i}", name=f"xkv{i}")
                  for i in range(N_IB)]
        for i in range(N_IB):
            nc.sync.dma_start(out=g_sb[i], in_=g[ts(i, P), :])
            # gather the 4 key stripes (cols 512k..512k+256 of xt) into a
            # dense [128, 1024] stripe-major kv tile
            nc.sync.dma_start(
                out=xkv_sb[i].rearrange("p (k c) -> p k c", c=256),
                in_=xt[ts(i, P), :].rearrange("p (k c) -> p k c", c=512)[:, :, 0:256])
        for i in range(N_IB):
            nc.sync.dma_start(out=wv_sb[i], in_=wvt[ts(i, P), :])
        for i in range(N_IB):
            nc.sync.dma_start(out=xt_sb[i][:, 0:NK], in_=xt[ts(i, P), 0:NK])
        for i in range(N_IB):
            nc.sync.dma_start(out=xt_sb[i][:, NK:NQ], in_=xt[ts(i, P), NK:NQ])

        # ---- projections ----
        # i-major emission in batches of 4 PSUM groups: each arriving DMA
        # block immediately feeds 4 matmuls, and consecutive matmuls with
        # the same stationary operand sit adjacent in the PE stream.
        def proj_phase(groups, dst, lhs_of, rhs_of):
            for gb in range(0, len(groups), 4):
                batch = groups[gb:gb + 4]
                accs = [ps.tile([P, RC], F32, tag="mm", name="acc_p")
                        for _ in batch]
                for i in range(N_IB):
                    for a, g in zip(accs, batch):
                        nc.tensor.matmul(a, lhsT=lhs_of(i, g),
                                         rhs=rhs_of(i, g),
                                         start=(i == 0), stop=(i == N_IB - 1))
                for a, g in zip(accs, batch):
                    nc.vector.tensor_copy(dst(g), a)

        mt_sb = [sb.tile([P, NK], F16, tag=f"mt{o}", name=f"mt{o}")
                 for o in range(N_IB)]
        proj_phase([(o, jc) for o in range(N_IB) for jc in range(NK // RC)],
                   dst=lambda gr: mt_sb[gr[0]][:, ts(gr[1], RC)],
                   lhs_of=lambda i, gr: g_sb[i][:, ts(gr[0], P)],
                   rhs_of=lambda i, gr: xkv_sb[i][:, ts(gr[1], RC)])

        v_sb = [sb.tile([P, D], F16, tag=f"vj{j}", name=f"vj{j}")
                for j in range(N_KT)]
        proj_phase([(j, dc) for j in range(N_KT) for dc in range(D // RC)],
                   dst=lambda gr: v_sb[gr[0]][:, ts(gr[1], RC)],
                   lhs_of=lambda i, gr: xkv_sb[i][:, ts(gr[0], P)],
                   rhs_of=lambda i, gr: wv_sb[i][:, ts(gr[1], RC)])

        # ---- attention ----
        # ST is emitted kt-major so the stationary K^T block is reused by
        # consecutive matmuls; PV uses P^T sub-blocks as the stationary
        # operand (V moving), which makes the row-sum l an extra N=1 matmul
        # on an already-loaded stationary and yields output in natural
        # [query, d] orientation.
        pt_tiles = {}

        def kept_rcs(kt):
            return [rc for rc in range(N_RC) if kt in _kept_kts(rc)]

        def emit_st(kt):
            rcs = kept_rcs(kt)
            accs = {rc: ps.tile([P, RC], F32, tag="mm", name="acc_st")
                    for rc in rcs}
            for o in range(N_IB):
                for rc in rcs:
                    nc.tensor.matmul(accs[rc],
                                     lhsT=mt_sb[o][:, ts(kt, P)],
                                     rhs=xt_sb[o][:, ts(rc, RC)],
                                     start=(o == 0), stop=(o == N_IB - 1))
            for rc in rcs:
                pt = pts.tile([P, RC], F16, tag=f"pt{kt}_{rc}",
                              name=f"pt{kt}_{rc}")
                nc.scalar.activation(pt, accs[rc],
                                     mybir.ActivationFunctionType.Exp,
                                     scale=SCALE)
                base = _mask_base(rc, kt)
                if base < P - 1:  # tile straddles the causal diagonal
                    nc.gpsimd.affine_select(
                        out=pt, in_=pt,
                        compare_op=mybir.AluOpType.is_ge, fill=0.0,
                        base=base, channel_multiplier=-1, pattern=[[1, RC]])
                pt_tiles[(kt, rc)] = pt

        def emit_pv(rc):
            kts = _kept_kts(rc)
            last = len(kts) - 1
            for rsub in range(RC // P):
                pos = [ps.tile([P, RC], F32, tag="mm", name="acc_pv")
                       for _ in range(D // RC)]
                pl = psl.tile([P, 1], F32, tag="lp", name="lp")
                for n, kt in enumerate(kts):
                    lhs = pt_tiles[(kt, rc)][:, ts(rsub, P)]
                    for dc, po in enumerate(pos):
                        nc.tensor.matmul(po, lhsT=lhs,
                                         rhs=v_sb[kt][:, ts(dc, RC)],
                                         start=(n == 0), stop=(n == last))
                    nc.tensor.matmul(pl, lhsT=lhs, rhs=ones,
                                     start=(n == 0), stop=(n == last))
                row = rc * RC + rsub * P
                for dc, po in enumerate(pos):
                    o_sb = outp.tile([P, RC], F32, tag="osb", name="osb")
                    nc.vector.tensor_copy(o_sb, po)
                    nc.sync.dma_start(out=ot[row:row + P, ts(dc, RC)],
                                      in_=o_sb)
                nc.vector.tensor_copy(l_sb[:, rc * 4 + rsub:rc * 4 + rsub + 1],
                                      pl)

        # software-pipelined emission: PV(rc) right after its last key tile
        emit_st(0)
        emit_st(1)
        emit_pv(0)
        emit_st(2)
        emit_st(3)
        emit_pv(1)
        emit_st(4)
        emit_st(5)
        emit_pv(2)
        emit_st(6)
        emit_st(7)
        emit_pv(3)
        nc.sync.dma_start(out=ls, in_=l_sb)


_NC_CACHE = {}


def _get_nc():
    if "nc" not in _NC_CACHE:
        nc = bacc.Bacc("TRN2", target_bir_lowering=False, debug=False,
                       enable_asserts=False, num_devices=N_CORES)
        xt = nc.dram_tensor("xt", [D, NQ], F16, kind="ExternalInput").ap()
        g = nc.dram_tensor("g", [D, D], F16, kind="ExternalInput").ap()
        wvt = nc.dram_tensor("wvt", [D, D], F16, kind="ExternalInput").ap()
        ot = nc.dram_tensor("ot", [NQ, D], F32, kind="ExternalOutput").ap()
        ls = nc.dram_tensor("ls", [P, N_RC * 4], F32, kind="ExternalOutput").ap()
        with tile.TileContext(nc) as tc:
            _emit(nc, tc, xt, g, wvt, ot, ls)
        nc.compile()
        _NC_CACHE["nc"] = nc
    return _NC_CACHE["nc"]


def make_in_maps(x, w_query, w_key, w_value):
    bf = np.float16
    wq32 = np.asarray(w_query, dtype=np.float32)
    wk32 = np.asarray(w_key, dtype=np.float32)
    # fold the Q and K projections: scores = x_kv (Wk^T Wq) x^T
    g_np = np.ascontiguousarray(wk32.T @ wq32).astype(bf)
    wvt = np.ascontiguousarray(np.asarray(w_value).T).astype(bf)
    in_maps = []
    for c in range(N_CORES):
        b, eta = c // 2, c % 2
        rows = (np.arange(NQ) + eta * STRIPE) % S  # cols past S wrap to junk
        xt_np = np.ascontiguousarray(np.asarray(x)[b, rows].T).astype(bf)
        in_maps.append({"xt": xt_np, "g": g_np, "wvt": wvt})
    return in_maps


def merge_outputs(results):
    num = np.zeros((B, S, D), np.float32)
    den = np.zeros((B, S), np.float32)
    for c in range(N_CORES):
        b, eta = c // 2, c % 2
        otc = np.asarray(results[c]["ot"])   # [NQ, D]
        # ls[p, col] holds l for query col c = col*128 + p
        lc = np.asarray(results[c]["ls"]).T.reshape(NQ)
        beta = eta * STRIPE
        nvalid = S - beta
        num[b, beta:] += otc[:nvalid]
        den[b, beta:] += lc[:nvalid]
    return (num / den[:, :, None]).astype(np.float32)


def kernel(x, w_query, w_key, w_value, _trace=False):
    nc = _get_nc()
    in_maps = make_in_maps(x, w_query, w_key, w_value)
    res = bass_utils.run_bass_kernel_spmd(
        nc, in_maps, core_ids=list(range(N_CORES)), trace=_trace)
    out = merge_outputs(res.results)
    if _trace:
        kernel.last_result = res
    return out


# revision 16
# speedup vs baseline: 1.0323x; 1.0223x over previous
"""Causal single-head attention (B=4, S=2048, D=1024) on 8 Trainium2 cores.

Sharding: 8 cores = (batch b, stripe-set eta). Core (b, eta) owns four
interleaved key stripes of 256 rows at global offsets 512k + 256*eta
(k = 0..3) of batch b, stored locally stripe-major (local key
ell in [256k, 256k+256) -> global 512k + 256*eta + ell%256). Queries are
fed "aligned" with base beta = 256*eta: query col c corresponds to global
row beta + c. Then the causal condition for local key tile kt vs query
chunk rc is c >= 512*(kt//2) + 128*(kt%2) + x — identical on every core,
so one SPMD program serves both stripe sets with a purely compile-time
block mask, and score blocks with kt >= 2*(rc+1) are skipped outright
(20 of 32 blocks kept vs 28 for a half-split). Cols past the sequence end
(eta=1, c >= 1792) compute junk that the host discards.

Softmax uses no max-subtraction (logits are O(1) for this problem:
|score/32| < ~4), so per-core partials are just num = exp(S)·V and
l = sum(exp(S)); the host merges halves with num/den addition and one
divide. This is mathematically identical to the reference softmax.

The Q and K projections are folded away algebraically: scores =
x_kv (Wk^T Wq) x^T, with G = Wk^T Wq precomputed on the HOST (weights
only). On-chip: M^T = G^T x_kv^T costs 1024*D^2 MACs — replacing the
2048*D^2 Q projection and 1024*D^2 K projection outright.

On-chip layout (all matmul operands fp16 — same PE rate as bf16 with 3
extra mantissa bits; every tensor here is O(10) so fp16 range is safe —
fp32 PSUM accumulation):
  xt = x_core^T [D=1024, NQ=2048]; xkv = stripe-gathered key cols of xt.
    MT = g.T @ xkv   [i', j]      V = xkv.T @ wvT   [j, d]
    ST = MT.T @ xt   (scores transposed: partition=key, free=query)
    PT = exp(ST/32) causally zeroed. PV runs with PT sub-blocks stationary
    and V moving: O[c, d] += PT_sub.T @ V, and the denominator comes free
    as an N=1 matmul on the same stationary: l = PT_sub.T @ 1s.
  Outputs per core: o [NQ, D] fp32 numerator, ls [128, 16] fp32 denom
  (query col c lives at ls[c % 128, c // 128]).
"""

import sys

sys.path.insert(0, "/opt/trn_rl_repo")

from contextlib import ExitStack

import ml_dtypes
import numpy as np

import concourse.bass as bass  # noqa: F401  (engine types resolve via bacc)
import concourse.mybir as mybir
import concourse.tile as tile
from concourse import bacc, bass_utils
from concourse.bass import ts

F16 = mybir.dt.float16  # same PE speed as bf16, 3 more mantissa bits
F32 = mybir.dt.float32

P = 128            # partitions
D = 1024           # model dim (d_in == d_out)
NQ = 2048          # query slots per core
NK = 1024          # keys per core
RC = 512           # query-chunk (matmul moving-dim) size
N_RC = NQ // RC    # 4
N_KT = NK // P     # 8 key tiles
N_IB = D // P      # 8 contraction blocks
SCALE = 1.0 / 32.0 # 1/sqrt(D)

N_CORES = 8
B, S = 4, 2048
STRIPE = 256


def _kept_kts(rc):
    # key tile kt (stripe k = kt//2) is visible to query chunk rc iff some
    # col c in [rc*512, rc*512+512) has c >= 512*(kt//2) + 128*(kt%2).
    return [kt for kt in range(N_KT) if kt < 2 * (rc + 1)]


def _mask_base(rc, kt):
    # keep when  y + 512*rc >= x + 512*(kt//2) + 128*(kt%2)
    return RC * rc - RC * (kt // 2) - P * (kt % 2)


def _emit(nc, tc, xt, g, wvt, ot, ls):
    with ExitStack() as ctx:
        sb = ctx.enter_context(tc.tile_pool(name="sb", bufs=1))
        pts = ctx.enter_context(tc.tile_pool(name="pts", bufs=1))
        outp = ctx.enter_context(tc.tile_pool(name="outp", bufs=4))
        ps = ctx.enter_context(tc.tile_pool(name="ps", bufs=7, space="PSUM"))
        psl = ctx.enter_context(tc.tile_pool(name="psl", bufs=1, space="PSUM"))

        ones = sb.tile([P, 1], F16, tag="ones", name="ones")
        nc.vector.memset(ones, 1.0)

        # HAM warm-up: ~6us of dummy matmuls that need no DMA, issued while
        # the NEFF preamble + first input loads run. They lift the PE clock
        # gate from 1.2 to 2.4 GHz before real matmuls arrive. The result is
        # parked in l_sb, whose every column is overwritten later.
        warm = sb.tile([P, RC], F16, tag="warm", name="warm")
        nc.vector.memset(warm, 0.0)
        l_sb = sb.tile([P, N_RC * 4], F32, tag="lsb", name="lsb")
        acc_w = ps.tile([P, RC], F32, tag="mm", name="acc_w")
        N_WARM = 12
        for w in range(N_WARM):
            nc.tensor.matmul(acc_w, lhsT=warm[:, 0:P], rhs=warm,
                             start=(w == 0), stop=(w == N_WARM - 1))
        nc.vector.tensor_copy(l_sb, acc_w[:, 0:N_RC * 4])

        # ---- input loads ----
        # Emission order = consumption order: MT needs only g + xkv
        # (4MB), so PE compute starts while wv/xt are still in flight.
        xt_sb = [sb.tile([P, NQ], F16, tag=f"xt{i}", name=f"xt{i}")
                 for i in range(N_IB)]
        g_sb = [sb.tile([P, D], F16, tag=f"g{i}", name=f"g{i}")
                for i in range(N_IB)]
        wv_sb = [sb.tile([P, D], F16, tag=f"wv{i}", name=f"wv{i}")
                 for i in range(N_IB)]
        xkv_sb = [sb.tile([P, NK], F16, tag=f"xkv{i}", name=f"xkv{i}")
                  for i in range(N_IB)]
        for i in range(N_IB):
            nc.sync.dma_start(out=g_sb[i], in_=g[ts(i, P), :])
            # gather the 4 key stripes (cols 512k..512k+256 of xt) into a
            # dense [128, 1024] stripe-major kv tile
            nc.sync.dma_start(
                out=xkv_sb[i].rearrange("p (k c) -> p k c", c=256),
                in_=xt[ts(i, P), :].rearrange("p (k c) -> p k c", c=512)[:, :, 0:256])
        for i in range(N_IB):
            nc.sync.dma_start(out=wv_sb[i], in_=wvt[ts(i, P), :])
        for i in range(N_IB):
            nc.sync.dma_start(out=xt_sb[i][:, 0:NK], in_=xt[ts(i, P), 0:NK])
        for i in range(N_IB):
            nc.sync.dma_start(out=xt_sb[i][:, NK:NQ], in_=xt[ts(i, P), NK:NQ])

        # ---- projections ----
        # i-major emission in batches of 4 PSUM groups: each arriving DMA
        # block immediately feeds 4 matmuls, and consecutive matmuls with
        # the same stationary operand sit adjacent in the PE stream.
        def proj_phase(groups, dst, lhs_of, rhs_of):
            for gb in range(0, len(groups), 4):
                batch = groups[gb:gb + 4]
                accs = [ps.tile([P, RC], F32, tag="mm", name="acc_p")
                        for _ in batch]
                for i in range(N_IB):
                    for a, g in zip(accs, batch):
                        nc.tensor.matmul(a, lhsT=lhs_of(i, g),
                                         rhs=rhs_of(i, g),
                                         start=(i == 0), stop=(i == N_IB - 1))
                for a, g in zip(accs, batch):
                    nc.vector.tensor_copy(dst(g), a)

        mt_sb = [sb.tile([P, NK], F16, tag=f"mt{o}", name=f"mt{o}")
                 for o in range(N_IB)]
        proj_phase([(o, jc) for o in range(N_IB) for jc in range(NK // RC)],
                   dst=lambda gr: mt_sb[gr[0]][:, ts(gr[1], RC)],
                   lhs_of=lambda i, gr: g_sb[i][:, ts(gr[0], P)],
                   rhs_of=lambda i, gr: xkv_sb[i][:, ts(gr[1], RC)])

        v_sb = [sb.tile([P, D], F16, tag=f"vj{j}", name=f"vj{j}")
                for j in range(N_KT)]
        proj_phase([(j, dc) for j in range(N_KT) for dc in range(D // RC)],
                   dst=lambda gr: v_sb[gr[0]][:, ts(gr[1], RC)],
                   lhs_of=lambda i, gr: xkv_sb[i][:, ts(gr[0], P)],
                   rhs_of=lambda i, gr: wv_sb[i][:, ts(gr[1], RC)])

        # ---- attention ----
        # ST is emitted kt-major so the stationary K^T block is reused by
        # consecutive matmuls; PV uses P^T sub-blocks as the stationary
        # operand (V moving), which makes the row-sum l an extra N=1 matmul
        # on an already-loaded stationary and yields output in natural
        # [query, d] orientation.
        pt_tiles = {}

        def kept_rcs(kt):
            return [rc for rc in range(N_RC) if kt in _kept_kts(rc)]

        def _trim(rc, kt):
            # odd boundary tile kt == 2rc+1: its first 128 query cols lie
            # strictly below the causal diagonal — skip them entirely.
            return P if kt == 2 * rc + 1 else 0

        def emit_st(kt):
            rcs = kept_rcs(kt)
            accs = {rc: ps.tile([P, RC], F32, tag="mm", name="acc_st")
                    for rc in rcs}
            for o in range(N_IB):
                for rc in rcs:
                    qo = _trim(rc, kt)
                    nc.tensor.matmul(accs[rc][:, qo:RC],
                                     lhsT=mt_sb[o][:, ts(kt, P)],
                                     rhs=xt_sb[o][:, rc * RC + qo:(rc + 1) * RC],
                                     start=(o == 0), stop=(o == N_IB - 1))
            for rc in rcs:
                qo = _trim(rc, kt)
                pt = pts.tile([P, RC], F16, tag=f"pt{kt}_{rc}",
                              name=f"pt{kt}_{rc}")
                nc.scalar.activation(pt[:, qo:RC], accs[rc][:, qo:RC],
                                     mybir.ActivationFunctionType.Exp,
                                     scale=SCALE)
                base = _mask_base(rc, kt) + qo
                if base < P - 1:  # tile straddles the causal diagonal
                    nc.gpsimd.affine_select(
                        out=pt[:, qo:RC], in_=pt[:, qo:RC],
                        compare_op=mybir.AluOpType.is_ge, fill=0.0,
                        base=base, channel_multiplier=-1,
                        pattern=[[1, RC - qo]])
                pt_tiles[(kt, rc)] = pt

        def emit_pv(rc):
            for rsub in range(RC // P):
                # the trimmed sub-block (kt == 2rc+1, rsub == 0) is all-zero
                kts = [kt for kt in _kept_kts(rc)
                       if not (rsub < _trim(rc, kt) // P)]
                last = len(kts) - 1
                pos = [ps.tile([P, RC], F32, tag="mm", name="acc_pv")
                       for _ in range(D // RC)]
                pl = psl.tile([P, 1], F32, tag="lp", name="lp")
                for n, kt in enumerate(kts):
                    lhs = pt_tiles[(kt, rc)][:, ts(rsub, P)]
                    for dc, po in enumerate(pos):
                        nc.tensor.matmul(po, lhsT=lhs,
                                         rhs=v_sb[kt][:, ts(dc, RC)],
                                         start=(n == 0), stop=(n == last))
                    nc.tensor.matmul(pl, lhsT=lhs, rhs=ones,
                                     start=(n == 0), stop=(n == last))
                row = rc * RC + rsub * P
                for dc, po in enumerate(pos):
                    o_sb = outp.tile([P, RC], F32, tag="osb", name="osb")
                    nc.vector.tensor_copy(o_sb, po)
                    nc.sync.dma_start(out=ot[row:row + P, ts(dc, RC)],
                                      in_=o_sb)
                nc.vector.tensor_copy(l_sb[:, rc * 4 + rsub:rc * 4 + rsub + 1],
                                      pl)

        # software-pipelined emission: PV(rc) right after its last key tile
        emit_st(0)
        emit_st(1)
        emit_pv(0)
        emit_st(2)
        emit_st(3)
        emit_pv(1)
        emit_st(4)
        emit_st(5)
        emit_pv(2)
        emit_st(6)
        emit_st(7)
        emit_pv(3)
        nc.sync.dma_start(out=ls, in_=l_sb)


_NC_CACHE = {}


def _get_nc():
    if "nc" not in _NC_CACHE:
        nc = bacc.Bacc("TRN2", target_bir_lowering=False, debug=False,
                       enable_asserts=False, num_devices=N_CORES)
        xt = nc.dram_tensor("xt", [D, NQ], F16, kind="ExternalInput").ap()
        g = nc.dram_tensor("g", [D, D], F16, kind="ExternalInput").ap()
        wvt = nc.dram_tensor("wvt", [D, D], F16, kind="ExternalInput").ap()
        ot = nc.dram_tensor("ot", [NQ, D], F32, kind="ExternalOutput").ap()
        ls = nc.dram_tensor("ls", [P, N_RC * 4], F32, kind="ExternalOutput").ap()
        with tile.TileContext(nc) as tc:
            _emit(nc, tc, xt, g, wvt, ot, ls)
        nc.compile()
        _NC_CACHE["nc"] = nc
    return _NC_CACHE["nc"]


def make_in_maps(x, w_query, w_key, w_value):
    bf = np.float16
    wq32 = np.asarray(w_query, dtype=np.float32)
    wk32 = np.asarray(w_key, dtype=np.float32)
    # fold the Q and K projections: scores = x_kv (Wk^T Wq) x^T
    g_np = np.ascontiguousarray(wk32.T @ wq32).astype(bf)
    wvt = np.ascontiguousarray(np.asarray(w_value).T).astype(bf)
    in_maps = []
    for c in range(N_CORES):
        b, eta = c // 2, c % 2
        rows = (np.arange(NQ) + eta * STRIPE) % S  # cols past S wrap to junk
        xt_np = np.ascontiguousarray(np.asarray(x)[b, rows].T).astype(bf)
        in_maps.append({"xt": xt_np, "g": g_np, "wvt": wvt})
    return in_maps


def merge_outputs(results):
    num = np.zeros((B, S, D), np.float32)
    den = np.zeros((B, S), np.float32)
    for c in range(N_CORES):
        b, eta = c // 2, c % 2
        otc = np.asarray(results[c]["ot"])   # [NQ, D]
        # ls[p, col] holds l for query col c = col*128 + p
        lc = np.asarray(results[c]["ls"]).T.reshape(NQ)
        beta = eta * STRIPE
        nvalid = S - beta
        num[b, beta:] += otc[:nvalid]
        den[b, beta:] += lc[:nvalid]
    return (num / den[:, :, None]).astype(np.float32)


def kernel(x, w_query, w_key, w_value, _trace=False):
    nc = _get_nc()
    in_maps = make_in_maps(x, w_query, w_key, w_value)
    res = bass_utils.run_bass_kernel_spmd(
        nc, in_maps, core_ids=list(range(N_CORES)), trace=_trace)
    out = merge_outputs(res.results)
    if _trace:
        kernel.last_result = res
    return out
